# revision 1
# baseline (speedup 1.0000x reference)
"""Trainium2 Bass kernel for nn_InvertibleFourierGaussianFilter.

The reference "Fourier Gaussian filter" (FWHM=1.0mm, spacing 1.0) is
mathematically a 5x5 separable Gaussian convolution (sigma ~ 0.4247 px,
taps at -2..2): reflect-padded by 2 rows (Y), circular by 2 cols (X).
The rfft2/irfft2 round trip in the reference is just its implementation.

Strategy: pure data parallel over the batch (16 views per core x 8
cores).  Host pads each view (reflect rows / wrap cols) so the device
kernel is a pure "valid" separable stencil.  Per 124-row chunk:

  - Y pass (all 5 taps) + the tiny X +-2 taps (coeff 1.35e-5) in one
    PSUM accumulation on the tensor engine: one fp32 banded matmul
    (exact) + one bf16 banded matmul whose operand x[c]+x[c+4] is
    pre-summed on the otherwise-idle gpsimd engine.
  - X center tap: scaled copy on the scalar engine (exact fp32).
  - X +-1 taps: tensor_tensor add + scalar_tensor_tensor FMA on the
    vector engine (exact fp32).

Total error vs the fp32 FFT reference ~2e-6 (bf16 on the 1.35e-5-weight
taps contributes ~1e-7; a ~1e-6 term comes from those taps also being
picked up, doubly attenuated, by the +-1 tap reads).
"""

import sys

import numpy as np

sys.path.insert(0, "/opt/trn_rl_repo")

import ml_dtypes
import concourse.bacc as bacc
import concourse.mybir as mybir
import concourse.tile as tile
from concourse.bass_utils import run_bass_kernel_spmd

N_CORES = 8
B_FULL, H, W = 128, 768, 1024
B_LOC = B_FULL // N_CORES  # 16 views per core
PAD = 2  # stencil radius
PADX = 4  # host wrap-padding per side along X (extra 2 for the +-2-tap reads)
HP, WP = H + 2 * PAD, W + 2 * PADX  # 772, 1032
WQ = W + PADX  # 1028: v4 wrap-pads 4 on the left only
WT = W + 2 * PAD  # 1028: width of the Y-pass intermediate t
CHUNK = 124  # output rows per full chunk (128 input rows incl. halo)

MODE = "v4"  # best measured: 638us HW, rel err 2.0e-6 (v1=738us, v2=660us, v3=679us)


def _taps() -> np.ndarray:
    """Normalized 1-D Gaussian taps, identical (up to f32 rounding) to the
    factorization of the reference's normalized 5x5 kernel."""
    sigma = 1.0 / 2.35482
    d = np.arange(-PAD, PAD + 1, dtype=np.float64)
    w = np.exp(-(d * d) / (2.0 * sigma * sigma))
    return (w / w.sum()).astype(np.float32)


def _banded(taps: np.ndarray) -> np.ndarray:
    """B[pi, po] = taps[pi - po]: matmul(lhsT=B[:cin,:cout], rhs=x) gives
    t[po, :] = sum_d taps[d] * x[po + d, :] (valid Y correlation)."""
    Bm = np.zeros((128, CHUNK), np.float32)
    for po in range(CHUNK):
        Bm[po : po + 2 * PAD + 1, po] = taps
    return Bm


def _row_chunks():
    """(r0, cin, cout) covering all 768 output rows of one padded view."""
    chunks = []
    r0 = 0
    while r0 < H:
        cout = min(CHUNK, H - r0)
        chunks.append((r0, cout + 2 * PAD, cout))
        r0 += cout
    return chunks


X_STRIPES = [(0, 512), (512, 512), (1024, WT - 1024)]


def _fp16_parts():
    """fp16 hi/lo splits of the taps and input scaling, chosen so every
    stationary value is a *normal* fp16 number (no subnormal-flush risk):
      B  ~= Bh + Bl            (Bh offset by -5e-4 so Bl ~ 5e-4, normal)
      x  ~= xh + xls * (1/256) (xls = (x - xh)*256 so its range is normal)
    Y result = Bh@xh + Bl@xh + (B/256)@xls, residual ~2^-22."""
    t64 = _taps().astype(np.float64)
    th = (t64 - 5e-4).astype(np.float16)
    tl = (t64 - th.astype(np.float64)).astype(np.float16)
    ts = (t64 / 256.0).astype(np.float16)
    ts[np.abs(ts.astype(np.float64)) < 6.2e-5] = 0  # drop subnormal entries
    return th, tl, ts


def _banded16(taps16) -> np.ndarray:
    Bm = np.zeros((128, CHUNK), np.float16)
    for po in range(CHUNK):
        Bm[po : po + 2 * PAD + 1, po] = taps16
    return Bm


W_DEV = 1021  # device computes out cols [0, 1021); host patches the last 3


def _build_v4():
    """v4: fp16 hi/lo Y-pass like v3, but the PSUM intermediate is one
    2-bank [124, 1024] tile (bufs=4 -> all 8 banks, deep PE pipelining)
    and the ragged 4-wide stripe is gone: the device produces out cols
    [0, 1021) and the host fills the last 3 columns exactly."""
    f32 = mybir.dt.float32
    f16 = mybir.dt.float16
    bf16 = mybir.dt.bfloat16
    wx = _taps()
    nc = bacc.Bacc("TRN2", target_bir_lowering=False, debug=False)
    xh_d = nc.dram_tensor("xh", [B_LOC, HP, WQ], f16, kind="ExternalInput")
    xl_d = nc.dram_tensor("xl", [B_LOC, HP, WQ], f16, kind="ExternalInput")
    bh_d = nc.dram_tensor("bh", [128, CHUNK], f16, kind="ExternalInput")
    bl_d = nc.dram_tensor("bl", [128, CHUNK], f16, kind="ExternalInput")
    bs_d = nc.dram_tensor("bs", [128, CHUNK], f16, kind="ExternalInput")
    bB = nc.dram_tensor("bB", [128, CHUNK], bf16, kind="ExternalInput")
    y = nc.dram_tensor("y", [B_LOC, H, W], f32, kind="ExternalOutput")

    with tile.TileContext(nc) as tc:
        with (
            tc.tile_pool(name="const", bufs=1) as cpool,
            tc.tile_pool(name="xin", bufs=6) as inpool,
            tc.tile_pool(name="ubf", bufs=4) as upool,
            tc.tile_pool(name="ps", bufs=4, space="PSUM") as pspool,
            tc.tile_pool(name="xout", bufs=4) as outpool,
        ):
            bh = cpool.tile([128, CHUNK], f16)
            bl = cpool.tile([128, CHUNK], f16)
            bs = cpool.tile([128, CHUNK], f16)
            bb = cpool.tile([128, CHUNK], bf16)
            nc.sync.dma_start(bh[:], bh_d[:])
            nc.sync.dma_start(bl[:], bl_d[:])
            nc.sync.dma_start(bs[:], bs_d[:])
            nc.sync.dma_start(bb[:], bB[:])
            for img in range(B_LOC):
                for r0, cin, cout in _row_chunks():
                    xh = inpool.tile([128, WQ], f16, tag="xh")
                    xl = inpool.tile([128, WQ], f16, tag="xl")
                    # SWDGE stripes a transfer across all 16 SDMA engines;
                    # the HWDGE ring only got 4 — split inputs across both.
                    nc.gpsimd.dma_start(xh[:cin, :], xh_d[img, r0 : r0 + cin, :])
                    nc.sync.dma_start(xl[:cin, :], xl_d[img, r0 : r0 + cin, :])
                    ubf = upool.tile([128, 1024], bf16, tag="ubf")
                    nc.gpsimd.tensor_tensor(
                        ubf[:cin, :],
                        xh[:cin, 0:1024],
                        xh[:cin, 4:1028],
                        op=mybir.AluOpType.add,
                    )
                    t = pspool.tile([CHUNK, 1024], f32, tag="ps")
                    for c0 in (0, 512):
                        nc.tensor.matmul(
                            t[:cout, c0 : c0 + 512],
                            bh[:cin, :cout],
                            xh[:cin, c0 + 2 : c0 + 2 + 512],
                            start=True,
                            stop=False,
                        )
                        nc.tensor.matmul(
                            t[:cout, c0 : c0 + 512],
                            bl[:cin, :cout],
                            xh[:cin, c0 + 2 : c0 + 2 + 512],
                            start=False,
                            stop=False,
                        )
                        nc.tensor.matmul(
                            t[:cout, c0 : c0 + 512],
                            bs[:cin, :cout],
                            xl[:cin, c0 + 2 : c0 + 2 + 512],
                            start=False,
                            stop=False,
                        )
                        nc.tensor.matmul(
                            t[:cout, c0 : c0 + 512],
                            bb[:cin, :cout],
                            ubf[:cin, c0 : c0 + 512],
                            start=False,
                            stop=True,
                        )
                    out = outpool.tile([CHUNK, W_DEV], f32, tag="xout")
                    nc.scalar.activation(
                        out[:cout, :],
                        t[:cout, 2 : 2 + W_DEV],
                        mybir.ActivationFunctionType.Copy,
                        scale=float(wx[2]),
                    )
                    for d in (1, 3):
                        nc.vector.scalar_tensor_tensor(
                            out[:cout, :],
                            t[:cout, d : d + W_DEV],
                            float(wx[1]),
                            out[:cout, :],
                            op0=mybir.AluOpType.mult,
                            op1=mybir.AluOpType.add,
                        )
                    nc.sync.dma_start(
                        y[img, r0 : r0 + cout, 0:W_DEV], out[:cout, :]
                    )
    nc.finalize()
    return nc


def _build_v3():
    """v3: like v2 but the Y pass runs as three fp16 matmuls (hi/lo
    decomposition, 1 cyc/row) instead of one fp32 matmul (4 cyc/row).
    Host supplies xh = fp16(x) and xls = fp16((x - xh)*256)."""
    f32 = mybir.dt.float32
    f16 = mybir.dt.float16
    bf16 = mybir.dt.bfloat16
    wx = _taps()
    nc = bacc.Bacc("TRN2", target_bir_lowering=False, debug=False)
    xh_d = nc.dram_tensor("xh", [B_LOC, HP, WP], f16, kind="ExternalInput")
    xl_d = nc.dram_tensor("xl", [B_LOC, HP, WP], f16, kind="ExternalInput")
    bh_d = nc.dram_tensor("bh", [128, CHUNK], f16, kind="ExternalInput")
    bl_d = nc.dram_tensor("bl", [128, CHUNK], f16, kind="ExternalInput")
    bs_d = nc.dram_tensor("bs", [128, CHUNK], f16, kind="ExternalInput")
    bB = nc.dram_tensor("bB", [128, CHUNK], bf16, kind="ExternalInput")
    y = nc.dram_tensor("y", [B_LOC, H, W], f32, kind="ExternalOutput")

    with tile.TileContext(nc) as tc:
        with (
            tc.tile_pool(name="const", bufs=1) as cpool,
            tc.tile_pool(name="xin", bufs=4) as inpool,
            tc.tile_pool(name="ubf", bufs=3) as upool,
            tc.tile_pool(name="ps", bufs=2, space="PSUM") as pspool,
            tc.tile_pool(name="xout", bufs=4) as outpool,
        ):
            bh = cpool.tile([128, CHUNK], f16)
            bl = cpool.tile([128, CHUNK], f16)
            bs = cpool.tile([128, CHUNK], f16)
            bb = cpool.tile([128, CHUNK], bf16)
            nc.sync.dma_start(bh[:], bh_d[:])
            nc.sync.dma_start(bl[:], bl_d[:])
            nc.sync.dma_start(bs[:], bs_d[:])
            nc.sync.dma_start(bb[:], bB[:])
            for img in range(B_LOC):
                for r0, cin, cout in _row_chunks():
                    xh = inpool.tile([128, WP], f16, tag="xh")
                    xl = inpool.tile([128, WP], f16, tag="xl")
                    nc.sync.dma_start(xh[:cin, :], xh_d[img, r0 : r0 + cin, :])
                    nc.sync.dma_start(xl[:cin, :], xl_d[img, r0 : r0 + cin, :])
                    ubf = upool.tile([128, WT], bf16, tag="ubf")
                    nc.gpsimd.tensor_tensor(
                        ubf[:cin, :],
                        xh[:cin, 0:WT],
                        xh[:cin, 4 : 4 + WT],
                        op=mybir.AluOpType.add,
                    )
                    t = pspool.tile([CHUNK, WT], f32, tag="ps")
                    for c0, w in X_STRIPES:
                        nc.tensor.matmul(
                            t[:cout, c0 : c0 + w],
                            bh[:cin, :cout],
                            xh[:cin, c0 + 2 : c0 + 2 + w],
                            start=True,
                            stop=False,
                        )
                        nc.tensor.matmul(
                            t[:cout, c0 : c0 + w],
                            bl[:cin, :cout],
                            xh[:cin, c0 + 2 : c0 + 2 + w],
                            start=False,
                            stop=False,
                        )
                        nc.tensor.matmul(
                            t[:cout, c0 : c0 + w],
                            bs[:cin, :cout],
                            xl[:cin, c0 + 2 : c0 + 2 + w],
                            start=False,
                            stop=False,
                        )
                        nc.tensor.matmul(
                            t[:cout, c0 : c0 + w],
                            bb[:cin, :cout],
                            ubf[:cin, c0 : c0 + w],
                            start=False,
                            stop=True,
                        )
                    out = outpool.tile([CHUNK, W], f32, tag="xout")
                    nc.scalar.activation(
                        out[:cout, :],
                        t[:cout, 2 : 2 + W],
                        mybir.ActivationFunctionType.Copy,
                        scale=float(wx[2]),
                    )
                    for d in (1, 3):
                        nc.vector.scalar_tensor_tensor(
                            out[:cout, :],
                            t[:cout, d : d + W],
                            float(wx[1]),
                            out[:cout, :],
                            op0=mybir.AluOpType.mult,
                            op1=mybir.AluOpType.add,
                        )
                    nc.sync.dma_start(y[img, r0 : r0 + cout, :], out[:cout, :])
    nc.finalize()
    return nc


def _build_v2(with_pm2: bool):
    """v2: PE does Y (fp32, exact) [+ X +-2 taps in bf16]; ACT does the X
    center tap; DVE does the X +-1 taps; gpsimd pre-sums the +-2 operand."""
    f32 = mybir.dt.float32
    bf16 = mybir.dt.bfloat16
    wx = _taps()
    nc = bacc.Bacc("TRN2", target_bir_lowering=False, debug=False)
    xp = nc.dram_tensor("xp", [B_LOC, HP, WP], f32, kind="ExternalInput")
    bY = nc.dram_tensor("bY", [128, CHUNK], f32, kind="ExternalInput")
    bB = nc.dram_tensor("bB", [128, CHUNK], bf16, kind="ExternalInput")
    y = nc.dram_tensor("y", [B_LOC, H, W], f32, kind="ExternalOutput")

    with tile.TileContext(nc) as tc:
        with (
            tc.tile_pool(name="const", bufs=1) as cpool,
            tc.tile_pool(name="xin", bufs=4) as inpool,
            tc.tile_pool(name="ubf", bufs=3) as upool,
            tc.tile_pool(name="ps", bufs=2, space="PSUM") as pspool,
            tc.tile_pool(name="xout", bufs=4) as outpool,
        ):
            bt = cpool.tile([128, CHUNK], f32)
            nc.sync.dma_start(bt[:], bY[:])
            if with_pm2:
                bb = cpool.tile([128, CHUNK], bf16)
                nc.sync.dma_start(bb[:], bB[:])
            for img in range(B_LOC):
                for r0, cin, cout in _row_chunks():
                    xin = inpool.tile([128, WP], f32, tag="xin")
                    nc.sync.dma_start(xin[:cin, :], xp[img, r0 : r0 + cin, :])
                    if with_pm2:
                        ubf = upool.tile([128, WT], bf16, tag="ubf")
                        nc.gpsimd.tensor_tensor(
                            ubf[:cin, :],
                            xin[:cin, 0:WT],
                            xin[:cin, 4 : 4 + WT],
                            op=mybir.AluOpType.add,
                        )
                    t = pspool.tile([CHUNK, WT], f32, tag="ps")
                    for c0, w in X_STRIPES:
                        nc.tensor.matmul(
                            t[:cout, c0 : c0 + w],
                            bt[:cin, :cout],
                            xin[:cin, c0 + 2 : c0 + 2 + w],
                            start=True,
                            stop=not with_pm2,
                        )
                        if with_pm2:
                            nc.tensor.matmul(
                                t[:cout, c0 : c0 + w],
                                bb[:cin, :cout],
                                ubf[:cin, c0 : c0 + w],
                                start=False,
                                stop=True,
                            )
                    out = outpool.tile([CHUNK, W], f32, tag="xout")
                    nc.scalar.activation(
                        out[:cout, :],
                        t[:cout, 2 : 2 + W],
                        mybir.ActivationFunctionType.Copy,
                        scale=float(wx[2]),
                    )
                    for d in (1, 3):
                        nc.vector.scalar_tensor_tensor(
                            out[:cout, :],
                            t[:cout, d : d + W],
                            float(wx[1]),
                            out[:cout, :],
                            op0=mybir.AluOpType.mult,
                            op1=mybir.AluOpType.add,
                        )
                    nc.sync.dma_start(y[img, r0 : r0 + cout, :], out[:cout, :])
    nc.finalize()
    return nc


def _build_v1():
    """v1 baseline: Y via fp32 banded matmul, X all 5 taps on ACT+DVE."""
    f32 = mybir.dt.float32
    wx = _taps()
    nc = bacc.Bacc("TRN2", target_bir_lowering=False, debug=False)
    xp = nc.dram_tensor("xp", [B_LOC, HP, WP], f32, kind="ExternalInput")
    bY = nc.dram_tensor("bY", [128, CHUNK], f32, kind="ExternalInput")
    nc.dram_tensor("bB", [128, CHUNK], mybir.dt.bfloat16, kind="ExternalInput")
    y = nc.dram_tensor("y", [B_LOC, H, W], f32, kind="ExternalOutput")

    with tile.TileContext(nc) as tc:
        with (
            tc.tile_pool(name="const", bufs=1) as cpool,
            tc.tile_pool(name="xin", bufs=4) as inpool,
            tc.tile_pool(name="ps", bufs=2, space="PSUM") as pspool,
            tc.tile_pool(name="xout", bufs=4) as outpool,
        ):
            bt = cpool.tile([128, CHUNK], f32)
            nc.sync.dma_start(bt[:], bY[:])
            for img in range(B_LOC):
                for r0, cin, cout in _row_chunks():
                    xin = inpool.tile([128, WP], f32, tag="xin")
                    nc.sync.dma_start(xin[:cin, :], xp[img, r0 : r0 + cin, :])
                    t = pspool.tile([CHUNK, WT], f32, tag="ps")
                    for c0, w in X_STRIPES:
                        nc.tensor.matmul(
                            t[:cout, c0 : c0 + w],
                            bt[:cin, :cout],
                            xin[:cin, c0 + 2 : c0 + 2 + w],
                            start=True,
                            stop=True,
                        )
                    out = outpool.tile([CHUNK, W], f32, tag="xout")
                    nc.scalar.activation(
                        out[:cout, :],
                        t[:cout, 2 : 2 + W],
                        mybir.ActivationFunctionType.Copy,
                        scale=float(wx[2]),
                    )
                    for d in (0, 1, 3, 4):
                        nc.vector.scalar_tensor_tensor(
                            out[:cout, :],
                            t[:cout, d : d + W],
                            float(wx[d]),
                            out[:cout, :],
                            op0=mybir.AluOpType.mult,
                            op1=mybir.AluOpType.add,
                        )
                    nc.sync.dma_start(y[img, r0 : r0 + cout, :], out[:cout, :])
    nc.finalize()
    return nc


_CACHE: dict = {}


def _get_program(mode: str):
    if mode not in _CACHE:
        if mode == "v1":
            _CACHE[mode] = _build_v1()
        elif mode == "d":
            _CACHE[mode] = _build_v2(with_pm2=False)
        elif mode == "v2":
            _CACHE[mode] = _build_v2(with_pm2=True)
        elif mode == "v3":
            _CACHE[mode] = _build_v3()
        elif mode == "v4":
            _CACHE[mode] = _build_v4()
        else:
            raise ValueError(mode)
    return _CACHE[mode]


def _patch_tail_cols(x: np.ndarray, out: np.ndarray):
    """Fill out[:, :, W_DEV:] (3 columns) exactly on the host."""
    t64 = _taps().astype(np.float64)
    k2 = np.outer(t64, t64)
    xr = np.pad(x, ((0, 0), (PAD, PAD), (0, 0)), mode="reflect").astype(np.float64)
    cols = np.arange(W_DEV, W)
    acc = np.zeros((x.shape[0], H, cols.size))
    for dy in range(2 * PAD + 1):
        for dx in range(2 * PAD + 1):
            src = (cols + dx - PAD) % W
            acc += k2[dy, dx] * xr[:, dy : dy + H, :][:, :, src]
    out[:, :, W_DEV:] = acc.astype(np.float32)


def _run(x, trace: bool = False, mode: str = MODE, **spmd_kwargs):
    x = np.ascontiguousarray(np.asarray(x, dtype=np.float32))
    assert x.shape == (B_FULL, H, W), x.shape
    if mode == "v4":
        xq = np.pad(x, ((0, 0), (PAD, PAD), (0, 0)), mode="reflect")
        xq = np.pad(xq, ((0, 0), (0, 0), (PADX, 0)), mode="wrap")
    else:
        xq = np.pad(x, ((0, 0), (PAD, PAD), (0, 0)), mode="reflect")
        xq = np.pad(xq, ((0, 0), (0, 0), (PADX, PADX)), mode="wrap")
    taps = _taps()
    Bm = _banded(taps)
    Bb = (Bm * (taps[0] / taps[2])).astype(ml_dtypes.bfloat16)
    if mode in ("v3", "v4"):
        th, tl, ts = _fp16_parts()
        xh = xq.astype(np.float16)
        xl = ((xq - xh.astype(np.float32)) * np.float32(256.0)).astype(np.float16)
        bh16, bl16, bs16 = _banded16(th), _banded16(tl), _banded16(ts)
        in_maps = [
            {
                "xh": np.ascontiguousarray(xh[i * B_LOC : (i + 1) * B_LOC]),
                "xl": np.ascontiguousarray(xl[i * B_LOC : (i + 1) * B_LOC]),
                "bh": bh16,
                "bl": bl16,
                "bs": bs16,
                "bB": Bb,
            }
            for i in range(N_CORES)
        ]
    else:
        in_maps = [
            {
                "xp": np.ascontiguousarray(xq[i * B_LOC : (i + 1) * B_LOC]),
                "bY": Bm,
                "bB": Bb,
            }
            for i in range(N_CORES)
        ]
    nc = _get_program(mode)
    res = run_bass_kernel_spmd(
        nc, in_maps, list(range(N_CORES)), trace=trace, **spmd_kwargs
    )
    out = np.concatenate([r["y"] for r in res.results], axis=0)
    out = np.ascontiguousarray(out.astype(np.float32, copy=False))
    if mode == "v4":
        _patch_tail_cols(x, out)
    return out, res


def kernel(x):
    out, _ = _run(x)
    return out



# revision 2
# speedup vs baseline: 3.6994x; 3.6994x over previous
"""Trainium2 Bass kernel for nn_InvertibleFourierGaussianFilter.

The reference "Fourier Gaussian filter" (FWHM=1.0mm, spacing 1.0) is
mathematically a 5x5 separable Gaussian convolution (sigma ~ 0.4247 px):
reflect-padded by 2 rows (Y), circular (X).  The +-2 taps have weight
1.36e-5, so a 3x3 separable stencil reproduces the output to ~5e-5 and
fp16 end-to-end lands at ~3e-4 relative error (tolerance is 2e-2).

The baseline (v4, 638us) was DMA-bound: fp16 hi/lo input pair + fp32
output = 101 MB/core, with the fp32 output funneled through the 4-engine
HWDGE ring (4 x 25 GB/s ~ 98.5% busy = critical path).

v5 strategy (data parallel, 16 views per core):
  - fp16 input AND output (51 MB/core total, vs HBM-per-NC ~358 GB/s).
  - Host packs 8 images side by side per DRAM row so each 128-row chunk
    is one contiguous ~2 MB transfer; all bulk DMA goes through SWDGE
    (nc.gpsimd) which stripes across all 16 SDMA engines.
  - Y pass: banded fp16 matmuls on PE (center-column band applied to x,
    neighbor-column band applied to u = xL + xR presummed on DVE), f32
    PSUM accumulation.
  - PSUM->SBUF eviction split between ACT (stripe 0) and DVE (stripe 1).
"""

import sys

import numpy as np

sys.path.insert(0, "/opt/trn_rl_repo")

import concourse.bacc as bacc
import concourse.mybir as mybir
import concourse.tile as tile
from concourse.bass_utils import run_bass_kernel_spmd

N_CORES = 8
B_FULL, H, W = 128, 768, 1024
B_LOC = B_FULL // N_CORES  # 16 views per core
G = 8  # images packed side-by-side per DRAM row
NG = B_LOC // G  # groups per core
WPAD = W + 2  # per-image row with 1 wrap column each side
PACKW = G * WPAD  # 8208 packed input row
OUTW = G * W  # 8192 packed output row
HP = H + 2  # reflect-1 rows
CHUNK = 126  # output rows per chunk (cin = 128 input rows)

MODE = "v5"


def _taps() -> np.ndarray:
    """Middle 3 taps of the reference's normalized 5-tap Gaussian."""
    sigma = 1.0 / 2.35482
    d = np.arange(-2, 3, dtype=np.float64)
    w = np.exp(-(d * d) / (2.0 * sigma * sigma))
    w /= w.sum()
    return w[1:4]


def _banded(taps3: np.ndarray, scale: float) -> np.ndarray:
    """B[pi, po] = taps3[pi - po] * scale: matmul(lhsT=B[:cin,:cout], rhs=x)
    gives t[po, :] = sum_d taps3[d] * x[po + d, :] (valid Y correlation)."""
    Bm = np.zeros((128, CHUNK), np.float16)
    t = (taps3.astype(np.float64) * scale).astype(np.float16)
    for po in range(CHUNK):
        Bm[po : po + 3, po] = t
    return Bm


def _row_chunks():
    chunks = []
    r0 = 0
    while r0 < H:
        cout = min(CHUNK, H - r0)
        chunks.append((r0, cout + 2, cout))
        r0 += cout
    return chunks


def _build_v5(out_dge: str = "gpsimd"):
    f16 = mybir.dt.float16
    f32 = mybir.dt.float32
    nc = bacc.Bacc("TRN2", target_bir_lowering=False, debug=False)
    xp_d = nc.dram_tensor("xp", [NG, HP, PACKW], f16, kind="ExternalInput")
    bc_d = nc.dram_tensor("bc", [128, CHUNK], f16, kind="ExternalInput")
    bn_d = nc.dram_tensor("bn", [128, CHUNK], f16, kind="ExternalInput")
    y = nc.dram_tensor("y", [NG, H, OUTW], f16, kind="ExternalOutput")

    with tile.TileContext(nc) as tc:
        with (
            tc.tile_pool(name="const", bufs=1) as cpool,
            tc.tile_pool(name="xin", bufs=3) as inpool,
            tc.tile_pool(name="u", bufs=3) as upool,
            tc.tile_pool(name="ps", bufs=4, space="PSUM") as pspool,
            tc.tile_pool(name="xout", bufs=3) as outpool,
        ):
            bc = cpool.tile([128, CHUNK], f16)
            bn = cpool.tile([128, CHUNK], f16)
            nc.sync.dma_start(bc[:], bc_d[:])
            nc.sync.dma_start(bn[:], bn_d[:])
            for g in range(NG):
                for r0, cin, cout in _row_chunks():
                    xin = inpool.tile([128, PACKW], f16, tag="xin")
                    nc.gpsimd.dma_start(xin[:cin, :], xp_d[g, r0 : r0 + cin, :])
                    out = outpool.tile([CHUNK, OUTW], f16, tag="xout")
                    for j in range(G):
                        x0 = j * WPAD
                        u = upool.tile([128, W], f16, tag="u")
                        nc.vector.tensor_tensor(
                            u[:cin, :],
                            xin[:cin, x0 : x0 + W],
                            xin[:cin, x0 + 2 : x0 + 2 + W],
                            op=mybir.AluOpType.add,
                        )
                        ps = pspool.tile([CHUNK, W], f32, tag="ps")
                        for c0 in (0, 512):
                            nc.tensor.matmul(
                                ps[:cout, c0 : c0 + 512],
                                bc[:cin, :cout],
                                xin[:cin, x0 + 1 + c0 : x0 + 1 + c0 + 512],
                                start=True,
                                stop=False,
                            )
                            nc.tensor.matmul(
                                ps[:cout, c0 : c0 + 512],
                                bn[:cin, :cout],
                                u[:cin, c0 : c0 + 512],
                                start=False,
                                stop=True,
                            )
                        o0 = j * W
                        nc.scalar.copy(
                            out[:cout, o0 : o0 + 512], ps[:cout, 0:512]
                        )
                        nc.vector.tensor_copy(
                            out[:cout, o0 + 512 : o0 + W], ps[:cout, 512:1024]
                        )
                    if out_dge == "gpsimd":
                        nc.gpsimd.dma_start(
                            y[g, r0 : r0 + cout, :], out[:cout, :]
                        )
                    else:
                        nc.sync.dma_start(y[g, r0 : r0 + cout, :], out[:cout, :])
    nc.finalize()
    return nc


_CACHE: dict = {}


def _get_program(mode: str):
    if mode not in _CACHE:
        if mode == "v5":
            _CACHE[mode] = _build_v5("gpsimd")
        elif mode == "v5h":
            _CACHE[mode] = _build_v5("sync")
        else:
            raise ValueError(mode)
    return _CACHE[mode]


def _pack_inputs(x: np.ndarray):
    """x [B_FULL, H, W] f32 -> per-core packed fp16 [NG, HP, PACKW]."""
    xh = x.astype(np.float16)
    xh = np.pad(xh, ((0, 0), (1, 1), (0, 0)), mode="reflect")
    xh = np.pad(xh, ((0, 0), (0, 0), (1, 1)), mode="wrap")  # [B, HP, WPAD]
    taps = _taps()
    bc = _banded(taps, float(taps[1]))
    bn = _banded(taps, float(taps[0]))
    in_maps = []
    for i in range(N_CORES):
        slab = xh[i * B_LOC : (i + 1) * B_LOC]  # [16, HP, WPAD]
        packed = np.ascontiguousarray(
            slab.reshape(NG, G, HP, WPAD).transpose(0, 2, 1, 3).reshape(
                NG, HP, PACKW
            )
        )
        in_maps.append({"xp": packed, "bc": bc, "bn": bn})
    return in_maps


def _unpack_output(res) -> np.ndarray:
    outs = []
    for r in res.results:
        yp = np.asarray(r["y"])  # [NG, H, OUTW] f16
        yp = yp.reshape(NG, H, G, W).transpose(0, 2, 1, 3).reshape(B_LOC, H, W)
        outs.append(yp)
    return np.concatenate(outs, axis=0).astype(np.float32)


def _run(x, trace: bool = False, mode: str = MODE, **spmd_kwargs):
    x = np.ascontiguousarray(np.asarray(x, dtype=np.float32))
    assert x.shape == (B_FULL, H, W), x.shape
    in_maps = _pack_inputs(x)
    nc = _get_program(mode)
    res = run_bass_kernel_spmd(
        nc, in_maps, list(range(N_CORES)), trace=trace, **spmd_kwargs
    )
    return _unpack_output(res), res


def kernel(x):
    out, _ = _run(x)
    return out


# revision 3
# speedup vs baseline: 3.8928x; 1.0523x over previous
"""Trainium2 Bass kernel for nn_InvertibleFourierGaussianFilter.

The reference "Fourier Gaussian filter" (FWHM=1.0mm, spacing 1.0) is
mathematically a 5x5 separable Gaussian convolution (sigma ~ 0.4247 px):
reflect-padded by 2 rows (Y), circular (X).  The +-2 taps have weight
1.36e-5, so a 3x3 separable stencil reproduces the output to ~5e-5 and
fp16 end-to-end lands at ~3e-4 relative error (tolerance is 2e-2).

The baseline (v4, 638us) was DMA-bound: fp16 hi/lo input pair + fp32
output = 101 MB/core, with the fp32 output funneled through the 4-engine
HWDGE ring (4 x 25 GB/s ~ 98.5% busy = critical path).

v5 strategy (data parallel, 16 views per core):
  - fp16 input AND output (51 MB/core total, vs HBM-per-NC ~358 GB/s).
  - Host packs 8 images side by side per DRAM row so each 128-row chunk
    is one contiguous ~2 MB transfer; all bulk DMA goes through SWDGE
    (nc.gpsimd) which stripes across all 16 SDMA engines.
  - Y pass: banded fp16 matmuls on PE (center-column band applied to x,
    neighbor-column band applied to u = xL + xR presummed on DVE), f32
    PSUM accumulation.
  - PSUM->SBUF eviction split between ACT (stripe 0) and DVE (stripe 1).
"""

import sys

import numpy as np

sys.path.insert(0, "/opt/trn_rl_repo")

import concourse.bacc as bacc
import concourse.mybir as mybir
import concourse.tile as tile
from concourse.bass_utils import run_bass_kernel_spmd

N_CORES = 8
B_FULL, H, W = 128, 768, 1024
B_LOC = B_FULL // N_CORES  # 16 views per core
G = 8  # images packed side-by-side per DRAM row
NG = B_LOC // G  # groups per core
WPAD = W + 2  # per-image row with 1 wrap column each side
PACKW = G * WPAD  # 8208 packed input row
OUTW = G * W  # 8192 packed output row
HP = H + 2  # reflect-1 rows
CHUNK = 126  # output rows per chunk (cin = 128 input rows)

MODE = "v5"


def _taps() -> np.ndarray:
    """Middle 3 taps of the reference's normalized 5-tap Gaussian."""
    sigma = 1.0 / 2.35482
    d = np.arange(-2, 3, dtype=np.float64)
    w = np.exp(-(d * d) / (2.0 * sigma * sigma))
    w /= w.sum()
    return w[1:4]


def _banded(taps3: np.ndarray, scale: float) -> np.ndarray:
    """B[pi, po] = taps3[pi - po] * scale: matmul(lhsT=B[:cin,:cout], rhs=x)
    gives t[po, :] = sum_d taps3[d] * x[po + d, :] (valid Y correlation)."""
    Bm = np.zeros((128, CHUNK), np.float16)
    t = (taps3.astype(np.float64) * scale).astype(np.float16)
    for po in range(CHUNK):
        Bm[po : po + 3, po] = t
    return Bm


def _row_chunks():
    chunks = []
    r0 = 0
    while r0 < H:
        cout = min(CHUNK, H - r0)
        chunks.append((r0, cout + 2, cout))
        r0 += cout
    return chunks


def _build_v5(
    out_dge: str = "gpsimd",
    in_bufs: int = 3,
    out_bufs: int = 3,
    dve_js: tuple = (),
):
    """dve_js: image slots within each 8-image chunk whose full PSUM
    eviction runs on DVE (the rest run on ACT).  Empty tuple = split every
    image's eviction 50/50 between ACT and DVE (the v5 behavior)."""
    f16 = mybir.dt.float16
    f32 = mybir.dt.float32
    nc = bacc.Bacc("TRN2", target_bir_lowering=False, debug=False)
    xp_d = nc.dram_tensor("xp", [NG, HP, PACKW], f16, kind="ExternalInput")
    bc_d = nc.dram_tensor("bc", [128, CHUNK], f16, kind="ExternalInput")
    bn_d = nc.dram_tensor("bn", [128, CHUNK], f16, kind="ExternalInput")
    y = nc.dram_tensor("y", [NG, H, OUTW], f16, kind="ExternalOutput")

    with tile.TileContext(nc) as tc:
        with (
            tc.tile_pool(name="const", bufs=1) as cpool,
            tc.tile_pool(name="xin", bufs=in_bufs) as inpool,
            tc.tile_pool(name="u", bufs=3) as upool,
            tc.tile_pool(name="ps", bufs=4, space="PSUM") as pspool,
            tc.tile_pool(name="xout", bufs=out_bufs) as outpool,
        ):
            bc = cpool.tile([128, CHUNK], f16)
            bn = cpool.tile([128, CHUNK], f16)
            nc.sync.dma_start(bc[:], bc_d[:])
            nc.sync.dma_start(bn[:], bn_d[:])
            for g in range(NG):
                for r0, cin, cout in _row_chunks():
                    xin = inpool.tile([128, PACKW], f16, tag="xin")
                    nc.gpsimd.dma_start(xin[:cin, :], xp_d[g, r0 : r0 + cin, :])
                    out = outpool.tile([CHUNK, OUTW], f16, tag="xout")
                    for j in range(G):
                        x0 = j * WPAD
                        u = upool.tile([128, W], f16, tag="u")
                        nc.vector.tensor_tensor(
                            u[:cin, :],
                            xin[:cin, x0 : x0 + W],
                            xin[:cin, x0 + 2 : x0 + 2 + W],
                            op=mybir.AluOpType.add,
                        )
                        ps = pspool.tile([CHUNK, W], f32, tag="ps")
                        for c0 in (0, 512):
                            nc.tensor.matmul(
                                ps[:cout, c0 : c0 + 512],
                                bc[:cin, :cout],
                                xin[:cin, x0 + 1 + c0 : x0 + 1 + c0 + 512],
                                start=True,
                                stop=False,
                            )
                            nc.tensor.matmul(
                                ps[:cout, c0 : c0 + 512],
                                bn[:cin, :cout],
                                u[:cin, c0 : c0 + 512],
                                start=False,
                                stop=True,
                            )
                        o0 = j * W
                        if not dve_js:
                            nc.scalar.copy(
                                out[:cout, o0 : o0 + 512], ps[:cout, 0:512]
                            )
                            nc.vector.tensor_copy(
                                out[:cout, o0 + 512 : o0 + W],
                                ps[:cout, 512:1024],
                            )
                        elif j in dve_js:
                            nc.vector.tensor_copy(
                                out[:cout, o0 : o0 + W], ps[:cout, :]
                            )
                        else:
                            nc.scalar.copy(
                                out[:cout, o0 : o0 + W], ps[:cout, :]
                            )
                    if out_dge == "gpsimd":
                        nc.gpsimd.dma_start(
                            y[g, r0 : r0 + cout, :], out[:cout, :]
                        )
                    else:
                        nc.sync.dma_start(y[g, r0 : r0 + cout, :], out[:cout, :])
    nc.finalize()
    return nc


_CACHE: dict = {}


def _get_program(mode: str):
    if mode not in _CACHE:
        if mode == "v5":
            _CACHE[mode] = _build_v5("gpsimd")
        elif mode == "v5h":
            _CACHE[mode] = _build_v5("sync")
        elif mode == "v6":
            _CACHE[mode] = _build_v5(
                "sync", in_bufs=4, out_bufs=4, dve_js=(3, 7)
            )
        else:
            raise ValueError(mode)
    return _CACHE[mode]


def _pack_inputs(x: np.ndarray):
    """x [B_FULL, H, W] f32 -> per-core packed fp16 [NG, HP, PACKW]."""
    xh = x.astype(np.float16)
    xh = np.pad(xh, ((0, 0), (1, 1), (0, 0)), mode="reflect")
    xh = np.pad(xh, ((0, 0), (0, 0), (1, 1)), mode="wrap")  # [B, HP, WPAD]
    taps = _taps()
    bc = _banded(taps, float(taps[1]))
    bn = _banded(taps, float(taps[0]))
    in_maps = []
    for i in range(N_CORES):
        slab = xh[i * B_LOC : (i + 1) * B_LOC]  # [16, HP, WPAD]
        packed = np.ascontiguousarray(
            slab.reshape(NG, G, HP, WPAD).transpose(0, 2, 1, 3).reshape(
                NG, HP, PACKW
            )
        )
        in_maps.append({"xp": packed, "bc": bc, "bn": bn})
    return in_maps


def _unpack_output(res) -> np.ndarray:
    outs = []
    for r in res.results:
        yp = np.asarray(r["y"])  # [NG, H, OUTW] f16
        yp = yp.reshape(NG, H, G, W).transpose(0, 2, 1, 3).reshape(B_LOC, H, W)
        outs.append(yp)
    return np.concatenate(outs, axis=0).astype(np.float32)


def _run(x, trace: bool = False, mode: str = MODE, **spmd_kwargs):
    x = np.ascontiguousarray(np.asarray(x, dtype=np.float32))
    assert x.shape == (B_FULL, H, W), x.shape
    in_maps = _pack_inputs(x)
    nc = _get_program(mode)
    res = run_bass_kernel_spmd(
        nc, in_maps, list(range(N_CORES)), trace=trace, **spmd_kwargs
    )
    return _unpack_output(res), res


def kernel(x):
    out, _ = _run(x)
    return out


# revision 6
# speedup vs baseline: 3.9690x; 1.0196x over previous
"""Trainium2 Bass kernel for nn_InvertibleFourierGaussianFilter.

The reference "Fourier Gaussian filter" (FWHM=1.0mm, spacing 1.0) is
mathematically a 5x5 separable Gaussian convolution (sigma ~ 0.4247 px):
reflect-padded by 2 rows (Y), circular (X).  The +-2 taps have weight
1.36e-5, so a 3x3 separable stencil reproduces the output to ~5e-5 and
fp16 end-to-end lands at ~3e-4 relative error (tolerance is 2e-2).

The baseline (v4, 638us) was DMA-bound: fp16 hi/lo input pair + fp32
output = 101 MB/core, with the fp32 output funneled through the 4-engine
HWDGE ring (4 x 25 GB/s ~ 98.5% busy = critical path).

v5 strategy (data parallel, 16 views per core):
  - fp16 input AND output (51 MB/core total, vs HBM-per-NC ~358 GB/s).
  - Host packs 8 images side by side per DRAM row so each 128-row chunk
    is one contiguous ~2 MB transfer; all bulk DMA goes through SWDGE
    (nc.gpsimd) which stripes across all 16 SDMA engines.
  - Y pass: banded fp16 matmuls on PE (center-column band applied to x,
    neighbor-column band applied to u = xL + xR presummed on DVE), f32
    PSUM accumulation.
  - PSUM->SBUF eviction split between ACT (stripe 0) and DVE (stripe 1).
"""

import sys

import numpy as np

sys.path.insert(0, "/opt/trn_rl_repo")

import concourse.bacc as bacc
import concourse.mybir as mybir
import concourse.tile as tile
from concourse.bass_utils import run_bass_kernel_spmd

N_CORES = 8
B_FULL, H, W = 128, 768, 1024
B_LOC = B_FULL // N_CORES  # 16 views per core
G = 8  # images packed side-by-side per DRAM row
NG = B_LOC // G  # groups per core
WPAD = W + 2  # per-image row with 1 wrap column each side
PACKW = G * WPAD  # 8208 packed input row
OUTW = G * W  # 8192 packed output row
HP = H + 2  # reflect-1 rows
CHUNK = 126  # output rows per chunk (cin = 128 input rows)

MODE = "v5"


def _taps() -> np.ndarray:
    """Middle 3 taps of the reference's normalized 5-tap Gaussian."""
    sigma = 1.0 / 2.35482
    d = np.arange(-2, 3, dtype=np.float64)
    w = np.exp(-(d * d) / (2.0 * sigma * sigma))
    w /= w.sum()
    return w[1:4]


def _banded(taps3: np.ndarray, scale: float) -> np.ndarray:
    """B[pi, po] = taps3[pi - po] * scale: matmul(lhsT=B[:cin,:cout], rhs=x)
    gives t[po, :] = sum_d taps3[d] * x[po + d, :] (valid Y correlation)."""
    Bm = np.zeros((128, CHUNK), np.float16)
    t = (taps3.astype(np.float64) * scale).astype(np.float16)
    for po in range(CHUNK):
        Bm[po : po + 3, po] = t
    return Bm


def _row_chunks():
    chunks = []
    r0 = 0
    while r0 < H:
        cout = min(CHUNK, H - r0)
        chunks.append((r0, cout + 2, cout))
        r0 += cout
    return chunks


def _build_v5(
    out_dge: str = "gpsimd",
    in_bufs: int = 3,
    out_bufs: int = 3,
    dve_js: tuple = (),
    in_dge: str = "gpsimd",
):
    """dve_js: image slots within each 8-image chunk whose full PSUM
    eviction runs on DVE (the rest run on ACT).  Empty tuple = split every
    image's eviction 50/50 between ACT and DVE (the v5 behavior)."""
    f16 = mybir.dt.float16
    f32 = mybir.dt.float32
    nc = bacc.Bacc("TRN2", target_bir_lowering=False, debug=False)
    xp_d = nc.dram_tensor("xp", [NG, HP, PACKW], f16, kind="ExternalInput")
    bc_d = nc.dram_tensor("bc", [128, CHUNK], f16, kind="ExternalInput")
    bn_d = nc.dram_tensor("bn", [128, CHUNK], f16, kind="ExternalInput")
    y = nc.dram_tensor("y", [NG, H, OUTW], f16, kind="ExternalOutput")

    with tile.TileContext(nc) as tc:
        with (
            tc.tile_pool(name="const", bufs=1) as cpool,
            tc.tile_pool(name="xin", bufs=in_bufs) as inpool,
            tc.tile_pool(name="u", bufs=3) as upool,
            tc.tile_pool(name="ps", bufs=4, space="PSUM") as pspool,
            tc.tile_pool(name="xout", bufs=out_bufs) as outpool,
        ):
            bc = cpool.tile([128, CHUNK], f16)
            bn = cpool.tile([128, CHUNK], f16)
            nc.sync.dma_start(bc[:], bc_d[:])
            nc.sync.dma_start(bn[:], bn_d[:])
            for g in range(NG):
                for r0, cin, cout in _row_chunks():
                    xin = inpool.tile([128, PACKW], f16, tag="xin")
                    in_eng = nc.gpsimd if in_dge == "gpsimd" else nc.sync
                    in_eng.dma_start(xin[:cin, :], xp_d[g, r0 : r0 + cin, :])
                    out = outpool.tile([CHUNK, OUTW], f16, tag="xout")
                    for j in range(G):
                        x0 = j * WPAD
                        u = upool.tile([128, W], f16, tag="u")
                        nc.vector.tensor_tensor(
                            u[:cin, :],
                            xin[:cin, x0 : x0 + W],
                            xin[:cin, x0 + 2 : x0 + 2 + W],
                            op=mybir.AluOpType.add,
                        )
                        ps = pspool.tile([CHUNK, W], f32, tag="ps")
                        for c0 in (0, 512):
                            nc.tensor.matmul(
                                ps[:cout, c0 : c0 + 512],
                                bc[:cin, :cout],
                                xin[:cin, x0 + 1 + c0 : x0 + 1 + c0 + 512],
                                start=True,
                                stop=False,
                            )
                            nc.tensor.matmul(
                                ps[:cout, c0 : c0 + 512],
                                bn[:cin, :cout],
                                u[:cin, c0 : c0 + 512],
                                start=False,
                                stop=True,
                            )
                        o0 = j * W
                        if not dve_js:
                            nc.scalar.copy(
                                out[:cout, o0 : o0 + 512], ps[:cout, 0:512]
                            )
                            nc.vector.tensor_copy(
                                out[:cout, o0 + 512 : o0 + W],
                                ps[:cout, 512:1024],
                            )
                        elif j in dve_js:
                            nc.vector.tensor_copy(
                                out[:cout, o0 : o0 + W], ps[:cout, :]
                            )
                        else:
                            nc.scalar.copy(
                                out[:cout, o0 : o0 + W], ps[:cout, :]
                            )
                    if out_dge == "gpsimd":
                        nc.gpsimd.dma_start(
                            y[g, r0 : r0 + cout, :], out[:cout, :]
                        )
                    else:
                        nc.sync.dma_start(y[g, r0 : r0 + cout, :], out[:cout, :])
    nc.finalize()
    return nc


_CACHE: dict = {}


def _get_program(mode: str):
    if mode not in _CACHE:
        if mode == "v5":
            _CACHE[mode] = _build_v5("gpsimd")
        elif mode == "v5h":
            _CACHE[mode] = _build_v5("sync")
        elif mode == "v6":
            _CACHE[mode] = _build_v5(
                "sync", in_bufs=4, out_bufs=4, dve_js=(3, 7)
            )
        elif mode == "v7":
            _CACHE[mode] = _build_v5(
                "gpsimd", in_bufs=4, out_bufs=4, dve_js=(3, 7), in_dge="sync"
            )
        else:
            raise ValueError(mode)
    return _CACHE[mode]


def _pack_inputs(x: np.ndarray):
    """x [B_FULL, H, W] f32 -> per-core packed fp16 [NG, HP, PACKW]."""
    xh = x.astype(np.float16)
    xh = np.pad(xh, ((0, 0), (1, 1), (0, 0)), mode="reflect")
    xh = np.pad(xh, ((0, 0), (0, 0), (1, 1)), mode="wrap")  # [B, HP, WPAD]
    taps = _taps()
    bc = _banded(taps, float(taps[1]))
    bn = _banded(taps, float(taps[0]))
    in_maps = []
    for i in range(N_CORES):
        slab = xh[i * B_LOC : (i + 1) * B_LOC]  # [16, HP, WPAD]
        packed = np.ascontiguousarray(
            slab.reshape(NG, G, HP, WPAD).transpose(0, 2, 1, 3).reshape(
                NG, HP, PACKW
            )
        )
        in_maps.append({"xp": packed, "bc": bc, "bn": bn})
    return in_maps


def _unpack_output(res) -> np.ndarray:
    outs = []
    for r in res.results:
        yp = np.asarray(r["y"])  # [NG, H, OUTW] f16
        yp = yp.reshape(NG, H, G, W).transpose(0, 2, 1, 3).reshape(B_LOC, H, W)
        outs.append(yp)
    return np.concatenate(outs, axis=0).astype(np.float32)


def _run(x, trace: bool = False, mode: str = MODE, **spmd_kwargs):
    x = np.ascontiguousarray(np.asarray(x, dtype=np.float32))
    assert x.shape == (B_FULL, H, W), x.shape
    in_maps = _pack_inputs(x)
    nc = _get_program(mode)
    res = run_bass_kernel_spmd(
        nc, in_maps, list(range(N_CORES)), trace=trace, **spmd_kwargs
    )
    return _unpack_output(res), res


def kernel(x):
    out, _ = _run(x)
    return out
